# revision 7
# baseline (speedup 1.0000x reference)
"""Trainium2 Bass kernel v2 for nn_DattaBotModel (pre-norm causal attention +
top-2-of-8 MoE FFN), expert-parallel across 8 NeuronCores with real token
dispatch.

Vs v1 (dense all-token MoE + 8MB AllReduce):
- WO partials ReduceScatter over token blocks (out 1MB) instead of the 8MB
  AllReduce; each core owns 256 tokens of h, computes rmsnorm+routing for
  them, and the routing matrix + bf16 normalized activations are AllGathered
  (64KB + 4MB) instead of a second full-h round trip.
- The FFN runs only on the <=768 tokens routed to this core's expert
  (dispatch via matmul prefix-sum ranks -> perm table -> indirect DMA row
  gather of token-major bf16 activations), a ~2.7x tensor-time cut over
  dense.
- MoE outputs scatter (indirect DMA) into a token-major bf16 buffer,
  ReduceScatter'd over token blocks; the owner core adds h exactly.
"""

import numpy as np
from contextlib import ExitStack

import concourse.bass as bass
import concourse.mybir as mybir
import concourse.tile as tile
from concourse.bass_utils import run_bass_kernel_spmd
from concourse.tile_rust import add_dep_helper

F32 = mybir.dt.float32
F32R = mybir.dt.float32r
BF16 = mybir.dt.bfloat16
I32 = mybir.dt.int32
U32 = mybir.dt.uint32
AF = mybir.ActivationFunctionType
OP = mybir.AluOpType

P = 128
B, S, D = 2, 1024, 1024
NH, HD = 16, 64
E, H = 8, 4096
T = B * S            # 2048 tokens
NCORES = 8
TB = T // NCORES     # 256 tokens owned per core
DT = D // P          # 8 feature tiles
HT = H // P          # 32 hidden tiles
NTB = T // 512       # 4 token blocks of 512
NTI = T // P         # 16 token tiles of 128
C = 768              # expert token capacity (max observed load 558)
CT = C // P          # 6 slot tiles
EPS = 1e-6
BIG = 4096.0         # slot offset pushing unselected tokens out of bounds

MAX_WAITS = 1  # this walrus build rejects >1 sync-wait on one instruction


def _split_waits(nc, limit=MAX_WAITS):
    """Move excess semaphore waits onto standalone NoOps before the owning
    instruction (same engine; waits are ge-conditions so order is free)."""
    n = 0
    for f in nc.m.functions:
        for b in f.blocks:
            out = []
            for inst in b.instructions:
                si = inst.sync_info
                if si is not None and si.on_wait and len(si.on_wait) > limit:
                    waits = list(si.on_wait)
                    sem = [w for w in waits if w.sync_type == "semaphore"]
                    other = [w for w in waits if w.sync_type != "semaphore"]
                    keep = limit - len(other)
                    assert keep >= 1
                    extra, kept = sem[:-keep], sem[-keep:]
                    for i in range(0, len(extra), limit):
                        nop = mybir.InstNoOp(
                            name=f"{inst.name}-wsplit{i}", ins=[], outs=[]
                        )
                        nop.engine = inst.engine
                        nop.sync_info = mybir.SyncInfo(
                            on_wait=list(extra[i : i + limit]), on_update=[]
                        )
                        out.append(nop)
                        n += 1
                    si.on_wait = other + kept
                out.append(inst)
            b.instructions = out
    return n


def r32(ap):
    return ap.bitcast(F32R)


class DmaMux:
    "Round-robin dma_start issue across engines to parallelize DGE issue."
    def __init__(self, nc, engines=None):
        self.engines = engines or [nc.sync, nc.scalar, nc.gpsimd]
        self.i = 0

    def __call__(self, out, in_):
        e = self.engines[self.i % len(self.engines)]
        self.i += 1
        return e.dma_start(out=out, in_=in_)


def build_bass():
    nc = bass.Bass()
    dp = nc.declare_dram_parameter

    xT = dp("xT", [D, T], F32, isOutput=False)              # x transposed
    xTm = dp("xTm", [D, TB], F32, isOutput=False)           # my token block of xT
    wqm = dp("wqm", [P, DT, P], F32R, isOutput=False)       # my-heads Q lhsT tiles
    wkm = dp("wkm", [P, DT, P], F32R, isOutput=False)
    wvm = dp("wvm", [P, DT, P], F32R, isOutput=False)
    wom = dp("wom", [P, D], F32R, isOutput=False)           # wo[:, myrows].T
    gwT = dp("gwT", [P, DT, E], F32, isOutput=False)        # gate_w.T tiles
    w1r = dp("w1r", [HT, P, DT, P], BF16, isOutput=False)   # fc1 lhsT tiles (bf16)
    w2r = dp("w2r", [DT, P, HT, P], BF16, isOutput=False)   # fc2 lhsT tiles (bf16)
    b1m = dp("b1m", [P, HT], F32, isOutput=False)
    b2m = dp("b2m", [P, DT], F32, isOutput=False)
    nwa = dp("nwa", [1, D], F32, isOutput=False)            # attn_norm_w row
    nwm = dp("nwm", [1, D], F32, isOutput=False)            # moe_norm_w row
    cosT = dp("cosT", [P, T], F32, isOutput=False)
    sinT = dp("sinT", [P, T], F32, isOutput=False)          # sign-folded
    mskd = dp("mskd", [P, P], F32, isOutput=False)          # k<=q 0/1
    ident = dp("ident", [P, P], F32, isOutput=False)
    identr = dp("identr", [P, P], F32R, isOutput=False)
    onesr = dp("onesr", [1, P], F32, isOutput=False)        # row of ones
    onesc = dp("onesc", [P, 1], F32, isOutput=False)        # col of ones
    sltm = dp("sltm", [P, P], F32, isOutput=False)          # strict lower tri (q<p)
    selc = dp("selc", [E, 1], F32, isOutput=False)          # one-hot(my expert) col
    tokids = dp("tokids", [P, NTI, 4], I32, isOutputFalse := False)  # 128*i + p
    outp = dp("outp", [TB, D], F32, isOutput=True)          # my 256 token rows
    import os
    KDEBUG = os.environ.get("KDEBUG", "0") == "1"
    if KDEBUG:
        dbg_perm = dp("dbg_perm", [C, 4], I32, isOutput=True)
        dbg_w = dp("dbg_w", [C, 4], F32, isOutput=True)
        dbg_myw = dp("dbg_myw", [1, T], F32, isOutput=True)
        dbg_hrs = dp("dbg_hrs", [D, TB], F32, isOutput=True)
        dbg_rag = dp("dbg_rag", [NCORES, E, TB], F32, isOutput=True)
        dbg_tn = dp("dbg_tn", [TB, D], BF16, isOutput=True)
        dbg_moers = dp("dbg_moers", [TB, D], BF16, isOutput=True)

    pT2 = nc.dram_tensor("pT2", [NCORES, D, TB], F32)       # WO partials, c-major
    h_rs = nc.dram_tensor("h_rs", [D, TB], F32)
    r_in = nc.dram_tensor("r_in", [E, TB], F32)
    r_ag = nc.dram_tensor("r_ag", [NCORES, E, TB], F32, addr_space="Shared")
    tn_in = nc.dram_tensor("tn_in", [TB, D], BF16)
    tn_ag = nc.dram_tensor("tn_ag", [T + P, D], BF16, addr_space="Shared")
    PBIG = int(BIG) + T + P                                 # slot table + junk zone
    perm = nc.dram_tensor("perm", [PBIG, 4], I32)           # slot -> token id
    wcomp = nc.dram_tensor("wcomp", [PBIG, 4], F32)         # slot -> routing weight
    moe_loc = nc.dram_tensor("moe_loc", [T + P, D], BF16)   # + junk row block
    moe_rs = nc.dram_tensor("moe_rs", [TB, D], BF16)

    groups = [list(range(NCORES))]
    dma = DmaMux(nc)

    tc = tile.TileContext(nc)
    tc.__enter__()
    ctx = ExitStack()
    if True:
        cpool = ctx.enter_context(tc.tile_pool(name="consts", bufs=1))

        # ---- persistent constants ----
        b1_sb = cpool.tile([P, HT], F32, tag="b1")
        dma(out=b1_sb[:], in_=b1m[:])
        b2_sb = cpool.tile([P, DT], F32, tag="b2")
        dma(out=b2_sb[:], in_=b2m[:])
        or_sb = cpool.tile([1, P], F32, tag="or")
        dma(out=or_sb[:], in_=onesr[:])
        oc_sb = cpool.tile([P, 1], F32, tag="oc")
        dma(out=oc_sb[:], in_=onesc[:])
        selc_sb = cpool.tile([E, 1], F32, tag="selc")
        dma(out=selc_sb[:], in_=selc[:])
        slt_sb = cpool.tile([P, P], F32, tag="slt")
        dma(out=slt_sb[:], in_=sltm[:])
        id_sb = cpool.tile([P, P], F32, tag="id")
        dma(out=id_sb[:], in_=ident[:])
        id_sbr = cpool.tile([P, P], F32R, tag="idr")
        dma(out=id_sbr[:], in_=identr[:])
        eps_sb = cpool.tile([1, 1], F32, tag="eps")
        nc.vector.memset(eps_sb[:], EPS)
        zc_sb = cpool.tile([P, 1], F32, tag="zc")
        nc.vector.memset(zc_sb[:], 0.0)

        # zero the moe scatter buffer + perm table early (overlaps attention)
        zb_sb = cpool.tile([P, D], BF16, tag="zb")
        nc.vector.memset(zb_sb[:], 0.0)
        for i in range(T // P):
            dma(out=moe_loc[i * P : (i + 1) * P, :], in_=zb_sb[:])
        dma(out=tn_ag[T : T + P, :], in_=zb_sb[:])
        zi_sb = cpool.tile([P, 4], I32, tag="zi")
        nc.vector.memset(zi_sb[:], T)
        for i in range(CT):
            dma(out=perm[i * P : (i + 1) * P, :], in_=zi_sb[:])
        zw_sb = cpool.tile([P, 4], F32, tag="zw")
        nc.vector.memset(zw_sb[:], 0.0)
        for i in range(CT):
            dma(out=wcomp[i * P : (i + 1) * P, :], in_=zw_sb[:])

        g5_ctx = ExitStack()
        g5_pool = g5_ctx.enter_context(tc.tile_pool(name="g5c", bufs=1))
        gw_sb = g5_pool.tile([P, DT, E], F32, tag="gw")
        dma(out=gw_sb[:], in_=gwT[:])
        nwm_sb = g5_pool.tile([1, D], F32, tag="nwm")
        dma(out=nwm_sb[:], in_=nwm[:])
        ao_ctx = ExitStack()
        ao_pool = ao_ctx.enter_context(tc.tile_pool(name="ao", bufs=1))
        aoT = ao_pool.tile([P, T], F32R, tag="aoT")
        wo_sb = ao_pool.tile([P, D], F32R, tag="wo")
        dma(out=wo_sb[:], in_=wom[:])
        qkv_ctx = ExitStack()
        qkv_pool = qkv_ctx.enter_context(tc.tile_pool(name="qkv", bufs=1))
        qT = qkv_pool.tile([P, T], F32R, tag="qT")
        kT = qkv_pool.tile([P, T], F32R, tag="kT")
        v_sb = qkv_pool.tile([P, NTI, 130], F32R, tag="v")
        cos_sb = qkv_pool.tile([P, T], F32, tag="cos")
        dma(out=cos_sb[:], in_=cosT[:])
        sin_sb = qkv_pool.tile([P, T], F32, tag="sin")
        dma(out=sin_sb[:], in_=sinT[:])
        msk_sb = qkv_pool.tile([P, P], F32, tag="msk")
        dma(out=msk_sb[:], in_=mskd[:])
        t_ctx = ExitStack()

        # =========== stage 1: t = rmsnorm(x) (feature-major) ===========
        tpool = t_ctx.enter_context(tc.tile_pool(name="tT", bufs=1))
        tT = [tpool.tile([P, T], F32R, tag=f"t{dt}", name=f"t{dt}") for dt in range(DT)]
        wq_sb = tpool.tile([P, DT, P], F32R, tag="wq")
        dma(out=wq_sb[:], in_=wqm[:])
        wk_sb = tpool.tile([P, DT, P], F32R, tag="wk")
        dma(out=wk_sb[:], in_=wkm[:])
        wv_sb = tpool.tile([P, DT, P], F32R, tag="wv")
        dma(out=wv_sb[:], in_=wvm[:])
        nwa_sb = tpool.tile([1, D], F32, tag="nwa")
        dma(out=nwa_sb[:], in_=nwa[:])
        with tc.tile_pool(name="s1", bufs=2) as s1, \
             tc.tile_pool(name="ps1", bufs=1, space="PSUM") as ps1, \
             tc.tile_pool(name="ps1b", bufs=2, space="PSUM") as ps1b:
            ssq = [ps1.tile([1, 512], F32, tag=f"ssq{tb}", name=f"ssq{tb}") for tb in range(NTB)]
            for dt in range(DT):
                xt = s1.tile([P, T], F32, tag="xt")
                dma(out=xt[:], in_=xT[dt * P : (dt + 1) * P, :])
                sq = s1.tile([P, T], F32, tag="sq")
                nc.vector.tensor_mul(out=sq[:], in0=xt[:], in1=xt[:])
                for tb in range(NTB):
                    nc.tensor.matmul(
                        ssq[tb][:], lhsT=oc_sb[:], rhs=sq[:, tb * 512 : (tb + 1) * 512],
                        start=(dt == 0), stop=(dt == DT - 1),
                    )
            r_row = s1.tile([1, T], F32, tag="rrow")
            for tb in range(NTB):
                srt = s1.tile([1, 512], F32, tag="srt")
                nc.scalar.activation(
                    out=srt[:], in_=ssq[tb][:], func=AF.Sqrt,
                    scale=1.0 / D, bias=eps_sb[:],
                )
                nc.vector.reciprocal(
                    out=r_row[0:1, tb * 512 : (tb + 1) * 512], in_=srt[:]
                )
            for dt in range(DT):
                xt = s1.tile([P, T], F32, tag="xt")
                dma(out=xt[:], in_=xT[dt * P : (dt + 1) * P, :])
                for tb in range(NTB):
                    cs = slice(tb * 512, (tb + 1) * 512)
                    rb = ps1b.tile([P, 512], F32, tag="rb")
                    nc.tensor.matmul(
                        rb[:], lhsT=nwa_sb[0:1, dt * P : (dt + 1) * P],
                        rhs=r_row[0:1, cs], start=True, stop=True,
                    )
                    nc.vector.tensor_mul(
                        out=tT[dt][:, cs], in0=xt[:, cs], in1=rb[:]
                    )

        # =========== stage 2: QKV (+RoPE on q,k) ===========
        with tc.tile_pool(name="ps2", bufs=3, space="PSUM") as ps2, \
             tc.tile_pool(name="s2", bufs=2) as s2:
            for dst, w in ((qT, wq_sb), (kT, wk_sb)):
                for tb in range(NTB):
                    cs = slice(tb * 512, (tb + 1) * 512)
                    pp = ps2.tile([P, 512], F32, tag="qk")
                    for dt in range(DT):
                        nc.tensor.matmul(
                            pp[:], lhsT=(w[:, dt, :]), rhs=(tT[dt][:, cs]),
                            start=(dt == 0), stop=(dt == DT - 1),
                        )
                    nc.scalar.copy(out=dst[:, cs], in_=pp[:])
            nc.vector.tensor_copy(out=v_sb[:, :, 64], in_=oc_sb[:].to_broadcast([P, NTI]))
            nc.vector.tensor_copy(out=v_sb[:, :, 129], in_=oc_sb[:].to_broadcast([P, NTI]))
            for ti in range(NTI):
                rs = slice(ti * P, (ti + 1) * P)
                pp = ps2.tile([P, P], F32, tag="v")
                for dt in range(DT):
                    nc.tensor.matmul(
                        pp[:], lhsT=(tT[dt][:, rs]), rhs=(wv_sb[:, dt, :]),
                        start=(dt == 0), stop=(dt == DT - 1),
                    )
                nc.vector.tensor_copy(out=v_sb[:, ti, 0:64], in_=pp[:, 0:64])
                nc.vector.tensor_copy(out=v_sb[:, ti, 65:129], in_=pp[:, 64:128])
            # RoPE: z' = z*cos + rot(z)*sin_signed
            for z in (qT, kT):
                rot = s2.tile([P, T], F32, tag="rot")
                for hh in range(2):
                    o = hh * 64
                    nc.vector.tensor_copy(out=rot[o : o + 32, :], in_=z[o + 32 : o + 64, :])
                    nc.vector.tensor_copy(out=rot[o + 32 : o + 64, :], in_=z[o : o + 32, :])
                zc = s2.tile([P, T], F32, tag="zc")
                nc.vector.tensor_mul(out=zc[:], in0=z[:], in1=cos_sb[:])
                nc.vector.tensor_mul(out=rot[:], in0=rot[:], in1=sin_sb[:])
                nc.vector.tensor_add(out=z[:], in0=zc[:], in1=rot[:])

        t_ctx.close()

        # =========== stage 3: attention, st-layout, fused rowsum ===========
        with tc.tile_pool(name="ps3", bufs=3, space="PSUM") as ps3, \
             tc.tile_pool(name="ps3a", bufs=2, space="PSUM") as ps3a, \
             tc.tile_pool(name="ps3b", bufs=1, space="PSUM") as ps3b, \
             tc.tile_pool(name="s3", bufs=5) as s3, \
             tc.tile_pool(name="s3b", bufs=2) as s3b:
            for b in range(B):
                for hh in range(2):
                    hr = slice(hh * 64, (hh + 1) * 64)
                    hv = slice(hh * 65, (hh + 1) * 65)
                    aops = []
                    for qb in range(2):
                        tb = 2 * b + qb
                        qcs = slice(tb * 512, (tb + 1) * 512)
                        ao = ps3a.tile([65, 512], F32, tag=f"ao{qb}")
                        nkt = 4 * (qb + 1)
                        for kt in range(nkt):
                            off = max(0, (kt - 4 * qb) * P)
                            gkt = b * 8 + kt
                            krs = slice(gkt * P, (gkt + 1) * P)
                            st = ps3.tile([P, 512], F32, tag="st")
                            nc.tensor.matmul(
                                st[:, off:512], lhsT=(kT[hr, krs]),
                                rhs=(qT[hr, tb * 512 + off : (tb + 1) * 512]),
                                start=True, stop=True,
                            )
                            ex = s3.tile([P, 512], F32R, tag="ex")
                            if off:
                                nc.vector.tensor_copy(
                                    out=ex[:, 0:off],
                                    in_=zc_sb[:].to_broadcast([P, off]),
                                )
                            nc.scalar.activation(
                                out=ex[:, off:512], in_=st[:, off:512],
                                func=AF.Exp, scale=0.125,
                            )
                            if kt >= 4 * qb:
                                nc.vector.tensor_mul(
                                    out=ex[:, off : off + P],
                                    in0=ex[:, off : off + P], in1=msk_sb[:],
                                )
                            nc.tensor.matmul(
                                ao[:], lhsT=(v_sb[:, gkt, hv]), rhs=(ex[:]),
                                start=(kt == 0), stop=(kt == nkt - 1),
                            )
                        aops.append((ao, qcs))
                    for qb, (ao, qcs) in enumerate(aops):
                        rs1 = s3b.tile([1, 512], F32, tag="rs1")
                        nc.scalar.copy(out=rs1[:], in_=ao[64:65, :])
                        rc1 = s3b.tile([1, 512], F32, tag="rc1")
                        nc.vector.reciprocal(out=rc1[:], in_=rs1[:])
                        nb = ps3b.tile([64, 512], F32, tag="nb")
                        nc.tensor.matmul(
                            nb[:], lhsT=or_sb[0:1, 0:64], rhs=rc1[:],
                            start=True, stop=True,
                        )
                        nbs = s3b.tile([64, 512], F32, tag="nbs")
                        nc.scalar.copy(out=nbs[:], in_=nb[:])
                        nc.vector.tensor_mul(out=aoT[hr, qcs], in0=ao[0:64, :], in1=nbs[:])

        qkv_ctx.close()

        # =========== stage 4: WO partials -> ReduceScatter (token blocks) ===
        with tc.tile_pool(name="ps4", bufs=2, space="PSUM") as ps4, \
             tc.tile_pool(name="s4", bufs=3) as s4:
            for dot in range(DT):
                for tb in range(NTB):
                    cs = slice(tb * 512, (tb + 1) * 512)
                    pp = ps4.tile([P, 512], F32, tag="p")
                    nc.tensor.matmul(
                        pp[:], lhsT=(wo_sb[:, dot * P : (dot + 1) * P]),
                        rhs=(aoT[:, cs]), start=True, stop=True,
                    )
                    sb_ = s4.tile([P, 512], F32, tag="p")
                    nc.scalar.copy(out=sb_[:], in_=pp[:])
                    for cb in range(2):
                        dma(
                            out=pT2[2 * tb + cb, dot * P : (dot + 1) * P, :],
                            in_=sb_[:, cb * 256 : (cb + 1) * 256],
                        )
            nc.gpsimd.collective_compute(
                "ReduceScatter", OP.add, replica_groups=groups,
                ins=[pT2[:, :, :]], outs=[h_rs[:, :]],
            )

        ao_ctx.close()

        # ===== stage 5: my h block, rmsnorm -> tn, routing, AllGathers =====
        s5p_ctx = ExitStack()
        s5p = s5p_ctx.enter_context(tc.tile_pool(name="s5p", bufs=1))
        h_sb = [s5p.tile([P, TB], F32, tag=f"h{dt}", name=f"h{dt}") for dt in range(DT)]
        with tc.tile_pool(name="s5", bufs=2) as s5, \
             tc.tile_pool(name="s5r", bufs=1) as s5r, \
             tc.tile_pool(name="ps5", bufs=1, space="PSUM") as ps5, \
             tc.tile_pool(name="ps5b", bufs=2, space="PSUM") as ps5b:
            ssq5 = ps5.tile([1, TB], F32, tag="ssq5")
            tn_sb = []
            for dt in range(DT):
                rws = slice(dt * P, (dt + 1) * P)
                hp = s5.tile([P, TB], F32, tag="hp")
                dma(out=hp[:], in_=h_rs[rws, :])
                xm = s5.tile([P, TB], F32, tag="xm")
                dma(out=xm[:], in_=xTm[rws, :])
                nc.vector.tensor_add(out=h_sb[dt][:], in0=hp[:], in1=xm[:])
                sq = s5.tile([P, TB], F32, tag="sq")
                nc.vector.tensor_mul(out=sq[:], in0=h_sb[dt][:], in1=h_sb[dt][:])
                nc.tensor.matmul(
                    ssq5[:], lhsT=oc_sb[:], rhs=sq[:],
                    start=(dt == 0), stop=(dt == DT - 1),
                )
            srt5 = s5r.tile([1, TB], F32, tag="srt5")
            nc.scalar.activation(
                out=srt5[:], in_=ssq5[:], func=AF.Sqrt, scale=1.0 / D, bias=eps_sb[:],
            )
            rr5 = s5r.tile([1, TB], F32, tag="rr5")
            nc.vector.reciprocal(out=rr5[:], in_=srt5[:])
            for dt in range(DT):
                rb = ps5b.tile([P, TB], F32, tag="rb")
                nc.tensor.matmul(
                    rb[:], lhsT=nwm_sb[0:1, dt * P : (dt + 1) * P],
                    rhs=rr5[:], start=True, stop=True,
                )
                tn_t = s5r.tile([P, TB], F32, tag=f"tn{dt}", name=f"tn5{dt}")
                nc.vector.tensor_mul(out=tn_t[:], in0=h_sb[dt][:], in1=rb[:])
                tn_sb.append(tn_t)
            # token-major bf16 copy of my tn block -> tn_in -> AllGather
            with tc.tile_pool(name="ps5t", bufs=2, space="PSUM") as ps5t, \
                 tc.tile_pool(name="s5t", bufs=2) as s5t:
                for ti in range(TB // P):
                    tb_sb = s5t.tile([P, D], BF16, tag="tb")
                    for dt in range(DT):
                        pt = ps5t.tile([P, P], F32, tag="pt")
                        nc.tensor.transpose(
                            out=pt[:], in_=tn_sb[dt][:, ti * P : (ti + 1) * P],
                            identity=id_sb[:],
                        )
                        nc.vector.tensor_copy(
                            out=tb_sb[:, dt * P : (dt + 1) * P], in_=pt[:]
                        )
                    dma(out=tn_in[ti * P : (ti + 1) * P, :], in_=tb_sb[:])
            # gate logits (token-major [128, E]) + top-2 softmax -> R row block
            with tc.tile_pool(name="ps5c", bufs=1, space="PSUM") as ps5c, \
                 tc.tile_pool(name="s5c", bufs=1) as s5c:
                log_ps = ps5c.tile([P, (TB // P) * E], F32, tag="log")
                for ti in range(TB // P):
                    for dt in range(DT):
                        nc.tensor.matmul(
                            log_ps[:, ti * E : (ti + 1) * E],
                            lhsT=tn_sb[dt][:, ti * P : (ti + 1) * P],
                            rhs=gw_sb[:, dt, :],
                            start=(dt == 0), stop=(dt == DT - 1),
                        )
                NT5 = TB // P  # 2
                log_sb = s5c.tile([P, NT5, E], F32, tag="logs")
                nc.scalar.copy(
                    out=log_sb[:].rearrange("p a b -> p (a b)"), in_=log_ps[:]
                )
                srt8 = s5c.tile([P, NT5, E], F32, tag="srt8")
                for ti in range(NT5):
                    nc.vector.max(out=srt8[:, ti], in_=log_sb[:, ti])
                m1 = srt8[:, :, 0]
                m2 = srt8[:, :, 1]
                dm = s5c.tile([P, NT5], F32, tag="dm")
                nc.vector.tensor_sub(out=dm[:], in0=m2, in1=m1)
                exr = s5c.tile([P, NT5], F32, tag="exr")
                nc.scalar.activation(out=exr[:], in_=dm[:], func=AF.Exp)
                den = s5c.tile([P, NT5], F32, tag="den")
                nc.vector.tensor_scalar_add(den[:], exr[:], 1.0)
                p1 = s5c.tile([P, NT5], F32, tag="p1")
                nc.vector.reciprocal(out=p1[:], in_=den[:])
                p2 = s5c.tile([P, NT5], F32, tag="p2")
                nc.vector.tensor_scalar(
                    out=p2[:], in0=p1[:], scalar1=-1.0, scalar2=-1.0,
                    op0=OP.mult, op1=OP.subtract,
                )
                wsum = s5c.tile([P, NT5, E], F32, tag="wsum")
                mk = s5c.tile([P, NT5, E], F32, tag="mk")
                nc.vector.tensor_tensor(
                    out=mk[:], in0=log_sb[:],
                    in1=srt8[:, :, 0:1].to_broadcast([P, NT5, E]), op=OP.is_equal,
                )
                nc.vector.tensor_tensor(
                    out=wsum[:], in0=mk[:],
                    in1=p1[:].unsqueeze(2).to_broadcast([P, NT5, E]), op=OP.mult,
                )
                nc.vector.tensor_tensor(
                    out=mk[:], in0=log_sb[:],
                    in1=srt8[:, :, 1:2].to_broadcast([P, NT5, E]), op=OP.is_equal,
                )
                nc.vector.scalar_tensor_tensor(
                    out=mk[:], in0=mk[:], scalar=1.0,
                    in1=p2[:].unsqueeze(2).to_broadcast([P, NT5, E]),
                    op0=OP.mult, op1=OP.mult,
                )
                nc.vector.tensor_add(out=wsum[:], in0=wsum[:], in1=mk[:])
                # R block [E, TB]: transpose wsum token tiles
                with tc.tile_pool(name="ps5d", bufs=2, space="PSUM") as ps5d:
                    rblk = s5c.tile([E, TB], F32, tag="rblk")
                    for ti in range(NT5):
                        rp = ps5d.tile([E, P], F32, tag="rp")
                        nc.tensor.transpose(
                            out=rp[:], in_=wsum[:, ti, :], identity=id_sb[:]
                        )
                        nc.vector.tensor_copy(
                            out=rblk[:, ti * P : (ti + 1) * P], in_=rp[:]
                        )
                    dma(out=r_in[:, :], in_=rblk[:])
            # routing matrix AllGather (64KB) then activations (4MB)
            i_agr = nc.gpsimd.collective_compute(
                "AllGather", OP.bypass, replica_groups=groups,
                ins=[r_in[:, :]], outs=[r_ag[:, :, :]],
            )
            i_agt = nc.gpsimd.collective_compute(
                "AllGather", OP.bypass, replica_groups=groups,
                ins=[tn_in[:, :]], outs=[tn_ag[0:T, :]],
            )
            add_dep_helper(i_agt.ins, i_agr.ins, reason="R gather first (small)")
        # precompute token-major h tiles (consumed after the moe RS)
        hT_ctx = ExitStack()
        hTp = hT_ctx.enter_context(tc.tile_pool(name="hTp", bufs=1))
        hT_sb = []
        with tc.tile_pool(name="ps9p", bufs=2, space="PSUM") as ps9p:
            for ti in range(TB // P):
                hT_t = hTp.tile([P, D], F32, tag=f"hT{ti}", name=f"hTt{ti}")
                for dt in range(DT):
                    pt = ps9p.tile([P, P], F32, tag="pt3")
                    nc.tensor.transpose(
                        out=pt[:], in_=h_sb[dt][:, ti * P : (ti + 1) * P],
                        identity=id_sb[:],
                    )
                    nc.vector.tensor_copy(
                        out=hT_t[:, dt * P : (dt + 1) * P], in_=pt[:]
                    )
                hT_sb.append(hT_t)

        # =========== stage 6: dispatch index build ===========
        d_ctx = ExitStack()
        d_sb = d_ctx.enter_context(tc.tile_pool(name="disp", bufs=1))
        with tc.tile_pool(name="s6", bufs=2) as s6, \
             tc.tile_pool(name="ps6", bufs=1, space="PSUM") as ps6:
            # my expert's weight row over all tokens: selc^T @ R_all
            rall = s6.tile([E, NCORES, TB], F32, tag="rall")
            for cb in range(NCORES):
                dma(out=rall[:, cb, :], in_=r_ag[cb])
            rallf = rall[:].rearrange("e c t -> e (c t)")
            myw_row = d_sb.tile([1, T], F32, tag="myw")
            for tb in range(NTB):
                cs = slice(tb * 512, (tb + 1) * 512)
                mp = ps6.tile([1, 512], F32, tag="mp")
                nc.tensor.matmul(
                    mp[:], lhsT=selc_sb[:], rhs=rallf[:, cs], start=True, stop=True,
                )
                nc.scalar.copy(out=myw_row[0:1, cs], in_=mp[:])
            # token-major myw columns and mask, prefix-sum ranks
            mrow = s6.tile([1, T], F32, tag="mrow")
            nc.vector.tensor_scalar(
                out=mrow[:], in0=myw_row[:], scalar1=0.0, scalar2=None, op0=OP.is_gt
            )
            mT = s6.tile([P, NTI], F32, tag="mT")
            dma(out=mT[:], in_=mrow[0:1, :].rearrange("a (i p) -> (a p) i", p=P))
            mywT = s6.tile([P, NTI], F32, tag="mywT")
            dma(out=mywT[:], in_=myw_row[0:1, :].rearrange("a (i p) -> (a p) i", p=P))
            rkp = ps6.tile([P, NTI], F32, tag="rkp")
            nc.tensor.matmul(rkp[:], lhsT=slt_sb[:], rhs=mT[:], start=True, stop=True)
            totc_p = ps6.tile([NTI, 1], F32, tag="totc")
            nc.tensor.matmul(totc_p[:], lhsT=mT[:], rhs=oc_sb[:], start=True, stop=True)
            totc_sb = s6.tile([NTI, 1], F32, tag="totcs")
            nc.scalar.copy(out=totc_sb[:], in_=totc_p[:])
            bop = ps6.tile([NTI, 1], F32, tag="bop")
            nc.tensor.matmul(
                bop[:], lhsT=slt_sb[0:NTI, 0:NTI], rhs=totc_sb[:],
                start=True, stop=True,
            )
            bo_sb = s6.tile([NTI, 1], F32, tag="bo")
            nc.scalar.copy(out=bo_sb[:], in_=bop[:])
            brp = ps6.tile([1, NTI], F32, tag="brp")
            nc.tensor.transpose(out=brp[:], in_=bo_sb[:], identity=id_sb[0:NTI, 0:NTI])
            br_sb = s6.tile([1, NTI], F32, tag="br")
            nc.scalar.copy(out=br_sb[:], in_=brp[:])
            bcp = ps6.tile([P, NTI], F32, tag="bcp")
            nc.tensor.matmul(bcp[:], lhsT=or_sb[:], rhs=br_sb[:], start=True, stop=True)
            rk_sb = s6.tile([P, NTI], F32, tag="rksb")
            nc.scalar.copy(out=rk_sb[:], in_=rkp[:])
            slot_sb = s6.tile([P, NTI], F32, tag="slot")
            nc.vector.tensor_tensor(out=slot_sb[:], in0=rk_sb[:], in1=bcp[:], op=OP.add)
            nc.vector.scalar_tensor_tensor(
                out=slot_sb[:], in0=mT[:], scalar=-BIG,
                in1=slot_sb[:], op0=OP.mult, op1=OP.add,
            )
            nc.vector.tensor_scalar_add(slot_sb[:], slot_sb[:], BIG)
            # scatter token ids into the perm table (unselected go OOB)
            for ti in range(NTI):
                scol = s6.tile([P, 1], I32, tag="scol")
                nc.vector.tensor_copy(out=scol[:], in_=slot_sb[:, ti : ti + 1])
                tokid = s6.tile([P, 4], I32, tag="tokid")
                dma(out=tokid[:], in_=tokids[:, ti, :])
                nc.gpsimd.indirect_dma_start(
                    out=perm[:, :],
                    out_offset=bass.IndirectOffsetOnAxis(ap=scol[:, :1], axis=0),
                    in_=tokid[:],
                    in_offset=None,
                )
                wv4 = s6.tile([P, 4], F32, tag="wv4")
                nc.vector.tensor_copy(
                    out=wv4[:], in_=mywT[:, ti : ti + 1].to_broadcast([P, 4])
                )
                nc.gpsimd.indirect_dma_start(
                    out=wcomp[:, :],
                    out_offset=bass.IndirectOffsetOnAxis(ap=scol[:, :1], axis=0),
                    in_=wv4[:],
                    in_offset=None,
                )

        # =========== stage 7: gather + FFN + scatter ===========
        f_ctx = ExitStack()
        fpool = f_ctx.enter_context(tc.tile_pool(name="ffn", bufs=1))
        pcol = []
        wcol = []
        with tc.tile_pool(name="s7", bufs=2) as s7, \
             tc.tile_pool(name="ps7", bufs=2, space="PSUM") as ps7:
            # slot -> token columns and slot weights
            for ti in range(CT):
                pc = fpool.tile([P, 1], I32, tag=f"pc{ti}", name=f"pc{ti}")
                dma(out=pc[:], in_=perm[ti * P : (ti + 1) * P, 0:1])
                pcol.append(pc)
                wc = fpool.tile([P, 1], F32, tag=f"wc{ti}", name=f"wc{ti}")
                dma(out=wc[:], in_=wcomp[ti * P : (ti + 1) * P, 0:1])
                wcol.append(wc)
            # gather compacted tokens (token-major bf16) and transpose to
            # feature-major f32 tn_g tiles
            tn_g = [
                fpool.tile([P, C], BF16, tag=f"tg{dt}", name=f"tg{dt}")
                for dt in range(DT)
            ]
            for ti in range(CT):
                gtok = s7.tile([P, D], BF16, tag="gtok")
                nc.gpsimd.indirect_dma_start(
                    out=gtok[:], out_offset=None,
                    in_=tn_ag[:, :],
                    in_offset=bass.IndirectOffsetOnAxis(ap=pcol[ti][:, :1], axis=0),
                )
                gf = s7.tile([P, D], F32R, tag="gf")
                nc.vector.tensor_copy(out=gf[:], in_=gtok[:])
                for dt in range(DT):
                    pt = ps7.tile([P, P], F32R, tag="pt")
                    nc.tensor.transpose(
                        out=pt[:], in_=gf[:, dt * P : (dt + 1) * P],
                        identity=id_sbr[:],
                    )
                    nc.vector.tensor_copy(
                        out=tn_g[dt][:, ti * P : (ti + 1) * P], in_=pt[:]
                    )

        # fc1 -> hid (bf16), fc2 -> eo -> transpose+scale -> scatter
        with tc.tile_pool(name="s8h", bufs=1) as s8h, \
             tc.tile_pool(name="s8w", bufs=3) as s8w, \
             tc.tile_pool(name="s8w2", bufs=3) as s8w2, \
             tc.tile_pool(name="s8o", bufs=2) as s8o, \
             tc.tile_pool(name="ps8a", bufs=3, space="PSUM") as ps8a, \
             tc.tile_pool(name="ps8b", bufs=3, space="PSUM") as ps8b, \
             tc.tile_pool(name="ps8c", bufs=2, space="PSUM") as ps8c:
            hid = []
            for ht in range(HT):
                w1_sb = s8w.tile([P, DT, P], BF16, tag="w1")
                dma(out=w1_sb[:], in_=w1r[ht])
                h_t = s8h.tile([P, C], BF16, tag=f"hh{ht}", name=f"hh{ht}")
                for half in range(2):
                    cs = slice(half * 384, (half + 1) * 384)
                    hp = ps8a.tile([P, 384], F32, tag="h")
                    for dt in range(DT):
                        nc.tensor.matmul(
                            hp[:], lhsT=(w1_sb[:, dt, :]), rhs=(tn_g[dt][:, cs]),
                            start=(dt == 0), stop=(dt == DT - 1),
                        )
                    nc.scalar.activation(
                        out=h_t[:, cs], in_=hp[:],
                        func=AF.Gelu, bias=b1_sb[:, ht : ht + 1],
                    )
                hid.append(h_t)
            eoT = s8o.tile([P, CT, D], BF16, tag="eoT", name="eoT")
            for dot in range(DT):
                w2a = s8w2.tile([P, HT // 2, P], BF16, tag="w2")
                dma(out=w2a[:], in_=w2r[dot, :, 0 : HT // 2, :])
                w2b = s8w2.tile([P, HT // 2, P], BF16, tag="w2")
                dma(out=w2b[:], in_=w2r[dot, :, HT // 2 :, :])
                eo_sb = s8o.tile([P, C], F32R, tag="eo")
                for half in range(2):
                    cs = slice(half * 384, (half + 1) * 384)
                    ep = ps8b.tile([P, 384], F32, tag="e")
                    for ht in range(HT):
                        w2t_ = w2a if ht < HT // 2 else w2b
                        nc.tensor.matmul(
                            ep[:], lhsT=(w2t_[:, ht % (HT // 2), :]),
                            rhs=(hid[ht][:, cs]),
                            start=(ht == 0), stop=(ht == HT - 1),
                        )
                    # + b2 (per-feature bias on the partition axis)
                    nc.scalar.activation(
                        out=eo_sb[:, cs], in_=ep[:],
                        func=AF.Identity, bias=b2_sb[:, dot : dot + 1],
                    )
                for ti in range(CT):
                    pt = ps8c.tile([P, P], F32R, tag="pt2")
                    nc.tensor.transpose(
                        out=pt[:],
                        in_=eo_sb[:, ti * P : (ti + 1) * P],
                        identity=id_sbr[:],
                    )
                    nc.vector.tensor_tensor(
                        out=eoT[:, ti, dot * P : (dot + 1) * P],
                        in0=pt[:], in1=wcol[ti][:].to_broadcast([P, P]),
                        op=OP.mult,
                    )
            # scatter weighted expert outputs to token-major moe buffer
            for ti in range(CT):
                nc.gpsimd.indirect_dma_start(
                    out=moe_loc[:, :],
                    out_offset=bass.IndirectOffsetOnAxis(ap=pcol[ti][:, :1], axis=0),
                    in_=eoT[:, ti, :],
                    in_offset=None,
                )
            nc.gpsimd.collective_compute(
                "ReduceScatter", OP.add, replica_groups=groups,
                ins=[moe_loc[0:T, :]], outs=[moe_rs[:, :]],
            )
        f_ctx.close()
        d_ctx.close()

        if KDEBUG:
            for i in range(CT):
                dma(out=dbg_perm[i * P : (i + 1) * P, :], in_=perm[i * P : (i + 1) * P, :])
                dma(out=dbg_w[i * P : (i + 1) * P, :], in_=wcomp[i * P : (i + 1) * P, :])
            dma(out=dbg_myw[:, :], in_=myw_row[:])
            for dt in range(DT):
                dma(out=dbg_hrs[dt * P : (dt + 1) * P, :], in_=h_rs[dt * P : (dt + 1) * P, :])
            for cb in range(NCORES):
                dma(out=dbg_rag[cb], in_=r_ag[cb])
            for i in range(TB // P):
                dma(out=dbg_tn[i * P : (i + 1) * P, :], in_=tn_ag[i * P : (i + 1) * P, :])
                dma(out=dbg_moers[i * P : (i + 1) * P, :], in_=moe_rs[i * P : (i + 1) * P, :])

        # =========== stage 8: out = h + moe (token-major) ===========
        with tc.tile_pool(name="s9", bufs=2) as s9:
            for ti in range(TB // P):
                mo = s9.tile([P, D], BF16, tag="mo")
                dma(out=mo[:], in_=moe_rs[ti * P : (ti + 1) * P, :])
                ot = s9.tile([P, D], F32, tag="ot")
                nc.vector.tensor_add(out=ot[:], in0=hT_sb[ti][:], in1=mo[:])
                dma(out=outp[ti * P : (ti + 1) * P, :], in_=ot[:])
        hT_ctx.close()

        s5p_ctx.close()
        g5_ctx.close()
    ctx.close()
    tc.__exit__(None, None, None)
    return nc


def host_inputs(x, attn_norm_w, wq, wk, wv, wo, moe_norm_w, gate_w, w1, b1, w2, b2):
    """Per-core input maps (shared arrays referenced, per-core weight shards)."""
    f = np.float32
    xT = np.ascontiguousarray(x.reshape(T, D).T, dtype=f)
    inv = 1.0 / (10000.0 ** (np.arange(0, HD, 2, dtype=np.float64) / HD))
    fr = np.arange(S, dtype=np.float64)[:, None] * inv
    emb = np.concatenate([fr, fr], -1)                     # [S, 64]
    cos_h = np.cos(emb).T.astype(f)                        # [64, S]
    sin_h = np.sin(emb).T.astype(f)
    sin_sgn = sin_h.copy()
    sin_sgn[0:32] *= -1.0
    cosT = np.tile(np.concatenate([cos_h, cos_h], 0), (1, B))
    sinT = np.tile(np.concatenate([sin_sgn, sin_sgn], 0), (1, B))
    mskd = (np.arange(P)[:, None] <= np.arange(P)[None, :]).astype(f)
    ident = np.eye(P, dtype=f)
    onesr = np.ones((1, P), f)
    onesc = np.ones((P, 1), f)
    sltm = (np.arange(P)[:, None] < np.arange(P)[None, :]).astype(f)
    nwa = np.ascontiguousarray(attn_norm_w[None, :], dtype=f)
    nwm = np.ascontiguousarray(moe_norm_w[None, :], dtype=f)
    gwT = np.ascontiguousarray(
        gate_w.T.reshape(DT, P, E).transpose(1, 0, 2), dtype=f
    )
    import ml_dtypes

    def to_bf16(a):
        return np.asarray(a, np.float32).astype(ml_dtypes.bfloat16)
    maps = []
    for c in range(NCORES):
        R = slice(P * c, P * (c + 1))
        selc = np.zeros((E, 1), f)
        selc[c, 0] = 1.0
        tokids = np.broadcast_to(
            (np.arange(NTI)[None, :, None] * P + np.arange(P)[:, None, None]),
            (P, NTI, 4),
        ).astype(np.int32)
        m = {
            "xT": xT, "xTm": np.ascontiguousarray(xT[:, TB * c : TB * (c + 1)]),
            "cosT": cosT, "sinT": sinT, "mskd": mskd, "ident": ident,
            "onesr": onesr, "onesc": onesc, "sltm": sltm, "identr": ident, "nwa": nwa, "nwm": nwm,
            "gwT": gwT, "selc": selc, "tokids": tokids,
            "wqm": np.ascontiguousarray(
                wq[R, :].T.reshape(DT, P, P).transpose(1, 0, 2), dtype=f),
            "wkm": np.ascontiguousarray(
                wk[R, :].T.reshape(DT, P, P).transpose(1, 0, 2), dtype=f),
            "wvm": np.ascontiguousarray(
                wv[R, :].T.reshape(DT, P, P).transpose(1, 0, 2), dtype=f),
            "wom": np.ascontiguousarray(wo[:, R].T, dtype=f),
            "w1r": to_bf16(np.ascontiguousarray(
                w1[c].T.reshape(DT, P, HT, P).transpose(2, 1, 0, 3), dtype=f)),
            "w2r": to_bf16(np.ascontiguousarray(
                w2[c].T.reshape(HT, P, DT, P).transpose(2, 1, 0, 3), dtype=f)),
            "b1m": np.ascontiguousarray(b1[c].reshape(HT, P).T, dtype=f),
            "b2m": np.ascontiguousarray(b2[c].reshape(DT, P).T, dtype=f),
        }
        maps.append(m)
    return maps


_CACHE = {}


def kernel(**inputs):
    inputs = {k: np.asarray(v) for k, v in inputs.items()}
    if "nc" not in _CACHE:
        _CACHE["nc"] = build_bass()
        _CACHE["nsplit"] = _split_waits(_CACHE["nc"])
    nc = _CACHE["nc"]
    in_maps = host_inputs(**inputs)
    res = run_bass_kernel_spmd(nc, in_maps, list(range(NCORES)))
    full = np.concatenate([res.results[c]["outp"] for c in range(NCORES)], 0)
    return np.ascontiguousarray(full).reshape(B, S, D).astype(np.float32)


if __name__ == "__main__":
    rng = np.random.default_rng(0)
    ins = {
        "x": rng.standard_normal((B, S, D), dtype=np.float32),
        "attn_norm_w": np.ones(D, np.float32),
        "wq": rng.standard_normal((D, D), dtype=np.float32) * 0.02,
        "wk": rng.standard_normal((D, D), dtype=np.float32) * 0.02,
        "wv": rng.standard_normal((D, D), dtype=np.float32) * 0.02,
        "wo": rng.standard_normal((D, D), dtype=np.float32) * 0.02,
        "moe_norm_w": np.ones(D, np.float32),
        "gate_w": rng.standard_normal((E, D), dtype=np.float32) * 0.02,
        "w1": rng.standard_normal((E, H, D), dtype=np.float32) * 0.02,
        "b1": np.zeros((E, H), np.float32),
        "w2": rng.standard_normal((E, D, H), dtype=np.float32) * 0.02,
        "b2": np.zeros((E, D), np.float32),
    }
    out = kernel(**ins)
    print(out.shape, out.dtype, np.abs(out).max())
